# revision 23
# baseline (speedup 1.0000x reference)
"""Trainium2 Bass kernel for nn_DA3CrossFrameCFDistanceLoss.

Strategy (8 NeuronCores):
  Phase 1 (data-parallel over batch x extra-frame shard):
    core c -> (b = c//4, shard s = c%4).  Host pre-normalizes both the ref
    rows and the extra-frame rows (folding the 1/||x|| scaling into the
    operands), scales by 64 and ships them as fp8e4m3, quartering HBM
    traffic.  Each core streams its frame transposed (extT [D, 4096] fp8)
    in 512-column blocks and runs DoubleRow fp8 matmuls (256-deep
    contraction per instruction, 2x PE rate) against the stationary ref
    block with fp32 PSUM accumulation.  Per block the DVE extracts top-8
    values + their indices per ref row directly from PSUM (max/max_index)
    and packs them (f32 bits + uint32) into one output row, DMA'd out per
    block.  The host merges the 8 blocks x 4 shards top-8s per row into
    the global top-4 and gathers the selected rows at full fp32 precision.
    (Selection is approximate only through the fp8 rounding of the
    similarity inputs - validated to move the final loss by ~5e-5
    relative against the 2e-2 tolerance.)
  Phase 2 (data-parallel over (batch, row-half, feature-half)):
    core c -> (b, h, dh).  Layout is transposed on host: d sits on the
    partition axis (4 chunks of 128), rows on the free axis.  The 19 KL
    "units" need Zt = sum exp(xt), Zs = sum exp(xs), num = sum exp(xt)*
    (xt - xs).  Using exp(a-b) = exp(a)*exp(-b), the ACT engine computes
    just 18 base exponentials (3 instructions per chunk); the DVE forms
    the per-unit products Pt, Ps and W = Pt*(xt-xs) as wide
    stride-0-broadcast tensor ops (7 instructions per chunk); the Pool
    engine supplies the base diffs; and the otherwise-idle PE performs
    all 57 per-unit reductions over d as ones-vector matmuls, one PSUM
    column per (chunk, tensor, unit) so no cross-instruction PSUM
    accumulation is needed (the bank-granular pending-zero of
    start_tensor_calc makes interleaved accumulation groups unsafe).
    Per-chunk results are copied and DMA'd out as soon as they finish;
    the host sums the chunk partials and evaluates kl = num/Zt - log Zt
    + log Zs, SmoothL1 and the weighted averaging.
"""

import numpy as np
import ml_dtypes

import concourse.bass as bass
from concourse import bacc
import concourse.mybir as mybir
from concourse import bass_utils
from concourse.tile import TileContext

# ---- problem constants (hardcoded from the nn.Module defaults) ----
B, V, P, D = 2, 8, 4096, 1024
EXTRA_FRAMES = [1, 3, 5, 7]
SHARED_TEACHER = [2, 4, 6]
SHARED_STUDENT = [1, 2, 3]
NUM_REF = 256
NUM_SHARED = 256
TOPK = 4
TEMP = 1.0
BETA = 0.5
N_CORES = 8

ES = P          # extra rows per shard (one frame per shard)
EB = 512        # phase-1 e-block size (one PSUM bank of fp32)
NBLK = ES // EB
DH = D // 2     # phase-2 feature half
NC4 = DH // 128  # phase-2 d chunks per core
N_UNITS = 19    # 3 d1 + 4 d2 + 12 d3

F32 = mybir.dt.float32
BF16 = mybir.dt.bfloat16
FP16 = mybir.dt.float16
FP8 = mybir.dt.float8e4
U16 = mybir.dt.uint16
U32 = mybir.dt.uint32

BF16_NP = ml_dtypes.bfloat16
FP8_NP = mybir.dt.np(FP8)

# phase-2 Zs unit permutation (Ps is built d2-first for AP adjacency)
ZS_PERM = [4, 5, 6, 0, 1, 2, 3] + list(range(7, 19))

_CACHE = {}

# Results of the most recent launches (exec_time_ns etc), for test harnesses.
LAST_PERF = {}


def _build_phase1():
    nc = bacc.Bacc("TRN2", target_bir_lowering=False, debug=False,
                   enable_asserts=False, num_devices=N_CORES)
    extT = nc.dram_tensor("extT", (D, ES), FP8, kind="ExternalInput").ap()
    refT = nc.dram_tensor("refT", (D, NUM_REF), FP8, kind="ExternalInput").ap()
    # packed per-block output: cols 0:8 top-8 values (f32 bits),
    # cols 8:16 their uint32 in-block indices
    out_o = nc.dram_tensor("out", (128, 2, NBLK, 16), U32,
                           kind="ExternalOutput").ap()

    extT_r = extT.rearrange("(k p) e -> p k e", p=128)
    refT_r = refT.rearrange("(k p) r -> p k r", p=128)
    DR = mybir.MatmulPerfMode.DoubleRow

    with TileContext(nc) as tc:
        with (
            tc.tile_pool(name="const", bufs=1) as const_pool,
            tc.tile_pool(name="xin", bufs=3) as xin_pool,
            tc.tile_pool(name="ps", bufs=6, space="PSUM") as ps_pool,
            tc.tile_pool(name="dum", bufs=1, space="PSUM") as dum_pool,
            tc.tile_pool(name="small", bufs=1) as small_pool,
        ):
            refT_sb = const_pool.tile([128, 8, NUM_REF], FP8)
            nc.scalar.dma_start(out=refT_sb, in_=refT_r)
            pk = small_pool.tile([128, 2, NBLK, 16], U32)
            dum = dum_pool.tile([128, 8], F32)

            for eb in range(NBLK):
                esl = slice(eb * EB, (eb + 1) * EB)
                xt = xin_pool.tile([128, 8, EB], FP8, tag="xt")
                # Split the first block's DMA by contraction halves so the
                # first matmuls can start as soon as k-chunks 0..3 land.
                if eb == 0:
                    nc.sync.dma_start(out=xt[:, 0:4], in_=extT_r[:, 0:4, esl])
                    nc.sync.dma_start(out=xt[:, 4:8], in_=extT_r[:, 4:8, esl])
                else:
                    nc.sync.dma_start(out=xt, in_=extT_r[:, :, esl])
                # A PE matmul may carry at most one semaphore wait (walrus
                # S3_LW limit).  Consume the xt-DMA dependencies with tiny
                # throwaway matmuls so the real matmuls only ever wait on one
                # source (the PSUM tile WAR release).
                nc.tensor.matmul(dum, lhsT=xt[:, 0, 0:128], rhs=xt[:, 0, 0:8],
                                 start=True, stop=True, skip_group_check=True)
                for m in range(2):
                    ps = ps_pool.tile([128, EB], F32, tag="ps", name="ps")
                    for k2 in range(4):
                        if eb == 0 and m == 0 and k2 == 2:
                            nc.tensor.matmul(dum, lhsT=xt[:, 4, 0:128],
                                             rhs=xt[:, 4, 0:8],
                                             start=True, stop=True,
                                             skip_group_check=True)
                        nc.tensor.matmul(
                            ps,
                            lhsT=refT_sb[:, 2 * k2:2 * k2 + 2,
                                         m * 128:(m + 1) * 128],
                            rhs=xt[:, 2 * k2:2 * k2 + 2, :],
                            start=(k2 == 0), stop=(k2 == 3),
                            perf_mode=DR,
                        )
                    nc.vector.max(out=pk[:, m, eb, 0:8].bitcast(F32), in_=ps)
                    nc.vector.max_index(out=pk[:, m, eb, 8:16],
                                        in_max=pk[:, m, eb, 0:8].bitcast(F32),
                                        in_values=ps)
                nc.scalar.dma_start(out=out_o[:, :, eb, :],
                                    in_=pk[:, :, eb, :])
    nc.compile()
    return nc


def _build_phase2():
    # Transposed layout: partition axis = d (NC4 chunks of 128), free = rows.
    # Input S slot order: rt=0, rs=1, sht=2..4, simh=5..8, shs=9..11.
    # Epos slots: rt=0, rs=1, sht=2..4 (instr a = exp S[0:5]),
    #             shs=5..7           (instr b = exp S[9:12])
    # Eneg slots (exp of -S[2:12]): sht=0..2, simh=3..6, shs=7..9
    nc = bacc.Bacc("TRN2", target_bir_lowering=False, debug=False,
                   enable_asserts=False, num_devices=N_CORES)
    S_in = nc.dram_tensor("sin", (128, NC4, 12, 128), FP16,
                          kind="ExternalInput").ap()
    ones_in = nc.dram_tensor("ones", (128, 1), FP16, kind="ExternalInput").ap()
    ZB_o = nc.dram_tensor("zb", (128, NC4, 3 * N_UNITS), F32,
                          kind="ExternalOutput").ap()

    Exp = mybir.ActivationFunctionType.Exp

    with TileContext(nc) as tc:
        with (
            tc.tile_pool(name="src", bufs=1) as src_pool,
            tc.tile_pool(name="zps", bufs=1, space="PSUM") as zps_pool,
        ):
            ones = src_pool.tile([128, 1], FP16)
            nc.sync.dma_start(out=ones, in_=ones_in)

            for c in range(NC4):
                S = src_pool.tile([128, 12, 128], FP16, tag=f"S{c}")
                nc.sync.dma_start(out=S, in_=S_in[:, c])

                Epos = src_pool.tile([128, 8, 128], FP16, tag=f"Ep{c}")
                nc.scalar.activation(Epos[:, 0:5], S[:, 0:5], Exp)
                nc.scalar.activation(Epos[:, 5:8], S[:, 9:12], Exp)
                Eneg = src_pool.tile([128, 10, 128], FP16, tag=f"En{c}")
                nc.scalar.activation(Eneg, S[:, 2:12], Exp, scale=-1.0)

                # base diffs for dap = xt - xs (on the Pool engine)
                rd = src_pool.tile([128, 1, 128], FP16, tag=f"rd{c}")
                nc.gpsimd.tensor_sub(rd, S[:, 0:1], S[:, 1:2])
                sd = src_pool.tile([128, 3, 128], FP16, tag=f"sd{c}")
                nc.gpsimd.tensor_sub(sd, S[:, 2:5], S[:, 9:12])
                dd1 = src_pool.tile([128, 3, 128], FP16, tag=f"dd{c}")
                nc.gpsimd.tensor_sub(dd1, rd.broadcast_to((128, 3, 128)), sd)

                def b3(ap, n):  # broadcast a [128,1,r] slice over n units
                    return ap.broadcast_to((128, n, 128))

                # products; Pt/W unit order: d1 j0..2, d2 k0..3, d3 (j,k)
                #           Ps   unit order: d2 k0..3, d1 j0..2, d3 (j,k)
                Pt = src_pool.tile([128, N_UNITS, 128], FP16, tag=f"Pt{c}")
                nc.vector.tensor_mul(Pt[:, 0:7], b3(Epos[:, 0:1], 7),
                                     Eneg[:, 0:7])
                sht_e = Epos[:, 2:5].rearrange("p j (o r) -> p j o r", o=1) \
                    .broadcast_to((128, 3, 4, 128))
                simh_e = Eneg[:, 3:7].rearrange("p (o k) r -> p o k r", o=1) \
                    .broadcast_to((128, 3, 4, 128))
                Pt_d3 = Pt[:, 7:19].rearrange("p (j k) r -> p j k r", j=3)
                nc.vector.tensor_mul(Pt_d3, sht_e, simh_e)

                Ps = src_pool.tile([128, N_UNITS, 128], FP16, tag=f"Qs{c}")
                nc.vector.tensor_mul(Ps[:, 0:7], b3(Epos[:, 1:2], 7),
                                     Eneg[:, 3:10])
                shs_e = Epos[:, 5:8].rearrange("p j (o r) -> p j o r", o=1) \
                    .broadcast_to((128, 3, 4, 128))
                Ps_d3 = Ps[:, 7:19].rearrange("p (j k) r -> p j k r", j=3)
                nc.vector.tensor_mul(Ps_d3, shs_e, simh_e)

                W = src_pool.tile([128, N_UNITS, 128], FP16, tag=f"W{c}")
                nc.gpsimd.tensor_mul(W[:, 0:3], Pt[:, 0:3], dd1)
                nc.gpsimd.tensor_mul(W[:, 3:7], Pt[:, 3:7], b3(rd, 4))
                sd_e = sd.rearrange("p j (o r) -> p j o r", o=1) \
                    .broadcast_to((128, 3, 4, 128))
                W_d3 = W[:, 7:19].rearrange("p (j k) r -> p j k r", j=3)
                nc.vector.tensor_mul(W_d3, Pt_d3, sd_e)

                # All 57 reductions over this chunk's d on the PE:
                # ones-vector matmuls, one PSUM column each (start+stop).
                Zc = zps_pool.tile([128, 3 * N_UNITS], F32, tag=f"Z{c}")
                for t, T in enumerate((Pt, Ps, W)):
                    for u in range(N_UNITS):
                        col = t * N_UNITS + u
                        nc.tensor.matmul(
                            Zc[:, col:col + 1],
                            lhsT=T[:, u, :],
                            rhs=ones,
                            start=True, stop=True,
                            skip_group_check=True,
                        )
                zsb = src_pool.tile([128, 3 * N_UNITS], F32, tag=f"zs{c}")
                nc.scalar.copy(zsb, Zc)
                nc.sync.dma_start(out=ZB_o[:, c], in_=zsb)
    nc.compile()
    return nc


def _get(name):
    if name not in _CACHE:
        _CACHE[name] = _build_phase1() if name == "p1" else _build_phase2()
    return _CACHE[name]


def kernel(**inputs):
    tf = np.ascontiguousarray(np.asarray(inputs["teacher_feats"], dtype=np.float32))
    sf = np.ascontiguousarray(np.asarray(inputs["student_feats"], dtype=np.float32))
    in_dtype = np.asarray(inputs["ref_perm"]).dtype
    ref_perm = np.asarray(inputs["ref_perm"]).astype(np.int64)[:NUM_REF]
    shared_perm = np.asarray(inputs["shared_perm"]).astype(np.int64)[:NUM_SHARED]
    assert in_dtype == np.int32

    # ---- host gathers + normalization (tiny) ----
    ref_t = tf[:, 0, ref_perm, :]                       # [B, 256, 1024]
    ref_s = sf[:, 0, ref_perm, :]
    rn = np.sqrt(np.einsum("brd,brd->br", ref_t, ref_t))[..., None]
    refn = ref_t / np.maximum(rn, 1e-12)
    # scale by 64 so fp8e4m3 operates in its normal range
    refTs = [np.ascontiguousarray((refn[b].T * 64.0).astype(FP8_NP))
             for b in range(B)]

    # ---- phase 1: sharded cosine-sim + per-block top-8 ----
    in_maps1 = []
    for c in range(N_CORES):
        b, s = divmod(c, 4)
        x = tf[b, EXTRA_FRAMES[s]]                      # [4096, 1024]
        nrm = np.sqrt(np.einsum("ed,ed->e", x, x))
        xn = x / np.maximum(nrm, 1e-12)[:, None]
        extT = np.ascontiguousarray((xn.T * 64.0).astype(FP8_NP))
        in_maps1.append({"extT": extT, "refT": refTs[b]})

    res1 = bass_utils.run_bass_kernel_spmd(
        _get("p1"), in_maps1, core_ids=list(range(N_CORES)))
    LAST_PERF["p1"] = res1

    # ---- host cross-block/cross-shard top-k merge ----
    gidx = np.zeros((B, NUM_REF, TOPK), dtype=np.int64)
    for b in range(B):
        vals_l, idxs_l = [], []
        for s in range(4):
            raw = np.ascontiguousarray(res1.results[b * 4 + s]["out"])
            vals = raw[..., 0:8].view(np.float32)            # [128,2,NBLK,8]
            bidx = raw[..., 8:16].astype(np.int64)
            gl = (bidx + (np.arange(NBLK) * EB)[None, None, :, None]
                  + s * ES)
            # row r = m*128 + p
            vals_l.append(vals.transpose(1, 0, 2, 3).reshape(NUM_REF, -1))
            idxs_l.append(gl.transpose(1, 0, 2, 3).reshape(NUM_REF, -1))
        vals = np.concatenate(vals_l, axis=1)
        idxs = np.concatenate(idxs_l, axis=1)
        order = np.argsort(-vals, axis=1, kind="stable")[:, :TOPK]
        gidx[b] = np.take_along_axis(idxs, order, axis=1)

    fr = np.asarray(EXTRA_FRAMES, dtype=np.int64)[gidx // P]
    pt = gidx % P
    sim_high = tf[np.arange(B)[:, None, None], fr, pt]  # [B, 256, 4, 1024]

    # ---- phase 2: distances ----
    sh_t = np.stack([tf[:, t, shared_perm, :] for t in SHARED_TEACHER], axis=1)
    sh_s = np.stack([sf[:, s, shared_perm, :] for s in SHARED_STUDENT], axis=1)

    def t_chunks(a):  # [128 rows, DH] -> [128 dpart, NC4, 128 rows]
        return a.T.reshape(NC4, 128, 128).transpose(1, 0, 2)

    ones_arr = np.ones((128, 1), dtype=np.float16)
    in_maps2 = []
    for c in range(N_CORES):
        b, h, dh = c >> 2, (c >> 1) & 1, c & 1
        rs = slice(h * 128, (h + 1) * 128)
        cs = slice(dh * DH, (dh + 1) * DH)
        # S slot order: rt, rs, sht0..2, simh0..3, shs0..2
        slots = ([ref_t[b, rs, cs], ref_s[b, rs, cs]]
                 + [sh_t[b, j, rs, cs] for j in range(3)]
                 + [sim_high[b, rs, k, cs] for k in range(4)]
                 + [sh_s[b, j, rs, cs] for j in range(3)])
        S = np.stack([t_chunks(a) for a in slots], axis=2)  # [128,NC4,12,128]
        in_maps2.append({"sin": np.ascontiguousarray(S.astype(np.float16)),
                         "ones": ones_arr})

    res2 = bass_utils.run_bass_kernel_spmd(
        _get("p2"), in_maps2, core_ids=list(range(N_CORES)))
    LAST_PERF["p2"] = res2

    # ---- host tail: kl + SmoothL1 + averaging ----
    s1 = s2 = s3 = 0.0
    for b in range(B):
        for h in range(2):
            z = (res2.results[b * 4 + h * 2 + 0]["zb"].astype(np.float64)
                 + res2.results[b * 4 + h * 2 + 1]["zb"].astype(np.float64)
                 ).sum(axis=1)
            Zt = z[:, 0:N_UNITS]
            Zs = z[:, N_UNITS:2 * N_UNITS][:, ZS_PERM]
            num = z[:, 2 * N_UNITS:3 * N_UNITS]          # [128, 19]
            kl = num / Zt - np.log(Zt) + np.log(Zs)
            akl = np.abs(kl)
            hub = np.where(akl < BETA, 0.5 * kl * kl / BETA, akl - 0.5 * BETA)
            s1 += hub[:, 0:3].sum()
            s2 += hub[:, 3:7].sum()
            s3 += hub[:, 7:19].sum()

    loss = (s1 / (3 * B * NUM_REF)
            + s2 / (B * NUM_REF * TOPK)
            + s3 / (3 * B * NUM_REF * TOPK))
    return np.float32(loss)


# revision 26
# speedup vs baseline: 1.0700x; 1.0700x over previous
"""Trainium2 Bass kernel for nn_DA3CrossFrameCFDistanceLoss.

Strategy (8 NeuronCores):
  Phase 1 (data-parallel over batch x extra-frame shard):
    core c -> (b = c//4, shard s = c%4).  Host pre-normalizes both the ref
    rows and the extra-frame rows (folding the 1/||x|| scaling into the
    operands), scales by 64 and ships them as fp8e4m3, quartering HBM
    traffic.  Each core streams its frame transposed (extT [D, 4096] fp8)
    in 512-column blocks and runs DoubleRow fp8 matmuls (256-deep
    contraction per instruction, 2x PE rate) against the stationary ref
    block with fp32 PSUM accumulation.  Per block the DVE extracts top-8
    values + their indices per ref row directly from PSUM (max/max_index)
    and packs them (f32 bits + uint32) into one output row, DMA'd out per
    block.  The host merges the 8 blocks x 4 shards top-8s per row into
    the global top-4 and gathers the selected rows at full fp32 precision.
    (Selection is approximate only through the fp8 rounding of the
    similarity inputs - validated to move the final loss by ~5e-5
    relative against the 2e-2 tolerance.)
  Phase 2 (data-parallel over (batch, row-half, feature-half)):
    core c -> (b, h, dh).  Layout is transposed on host: d sits on the
    partition axis (4 chunks of 128), rows on the free axis.  The 19 KL
    "units" need Zt = sum exp(xt), Zs = sum exp(xs), num = sum exp(xt)*
    (xt - xs).  Using exp(a-b) = exp(a)*exp(-b), the ACT engine computes
    just 18 base exponentials (3 instructions per chunk); the DVE forms
    the per-unit products Pt, Ps and W = Pt*(xt-xs) as wide
    stride-0-broadcast tensor ops (7 instructions per chunk); the Pool
    engine supplies the base diffs; and the otherwise-idle PE performs
    all 57 per-unit reductions over d as ones-vector matmuls, one PSUM
    column per (chunk, tensor, unit) so no cross-instruction PSUM
    accumulation is needed (the bank-granular pending-zero of
    start_tensor_calc makes interleaved accumulation groups unsafe).
    Per-chunk results are copied and DMA'd out as soon as they finish;
    the host sums the chunk partials and evaluates kl = num/Zt - log Zt
    + log Zs, SmoothL1 and the weighted averaging.
"""

import numpy as np
import ml_dtypes

import concourse.bass as bass
from concourse import bacc
import concourse.mybir as mybir
from concourse import bass_utils
from concourse.tile import TileContext

# ---- problem constants (hardcoded from the nn.Module defaults) ----
B, V, P, D = 2, 8, 4096, 1024
EXTRA_FRAMES = [1, 3, 5, 7]
SHARED_TEACHER = [2, 4, 6]
SHARED_STUDENT = [1, 2, 3]
NUM_REF = 256
NUM_SHARED = 256
TOPK = 4
TEMP = 1.0
BETA = 0.5
N_CORES = 8

ES = P          # extra rows per shard (one frame per shard)
EB = 512        # phase-1 e-block size (one PSUM bank of fp32)
NBLK = ES // EB
DH = D // 2     # phase-2 feature half
NC4 = DH // 128  # phase-2 d chunks per core
N_UNITS = 19    # 3 d1 + 4 d2 + 12 d3

F32 = mybir.dt.float32
BF16 = mybir.dt.bfloat16
FP16 = mybir.dt.float16
FP8 = mybir.dt.float8e4
U16 = mybir.dt.uint16
U32 = mybir.dt.uint32

BF16_NP = ml_dtypes.bfloat16
FP8_NP = mybir.dt.np(FP8)

# phase-2 Zs unit permutation (Ps is built d2-first for AP adjacency)
ZS_PERM = [4, 5, 6, 0, 1, 2, 3] + list(range(7, 19))

_CACHE = {}

# Results of the most recent launches (exec_time_ns etc), for test harnesses.
LAST_PERF = {}


def _build_phase1():
    nc = bacc.Bacc("TRN2", target_bir_lowering=False, debug=False,
                   enable_asserts=False, num_devices=N_CORES)
    extT = nc.dram_tensor("extT", (D, ES), FP8, kind="ExternalInput").ap()
    refT = nc.dram_tensor("refT", (D, NUM_REF), FP8, kind="ExternalInput").ap()
    # packed per-block output: cols 0:8 top-8 values (f32 bits),
    # cols 8:16 their uint32 in-block indices
    out_o = nc.dram_tensor("out", (128, 2, NBLK, 16), U32,
                           kind="ExternalOutput").ap()

    extT_r = extT.rearrange("(k p) e -> p k e", p=128)
    refT_r = refT.rearrange("(k p) r -> p k r", p=128)
    DR = mybir.MatmulPerfMode.DoubleRow

    with TileContext(nc) as tc:
        with (
            tc.tile_pool(name="const", bufs=1) as const_pool,
            tc.tile_pool(name="xin", bufs=3) as xin_pool,
            tc.tile_pool(name="ps", bufs=6, space="PSUM") as ps_pool,
            tc.tile_pool(name="dum", bufs=1, space="PSUM") as dum_pool,
            tc.tile_pool(name="small", bufs=1) as small_pool,
        ):
            refT_sb = const_pool.tile([128, 8, NUM_REF], FP8)
            nc.sync.dma_start(out=refT_sb, in_=refT_r)
            pk = small_pool.tile([128, 2, NBLK, 16], U32)
            dum = dum_pool.tile([128, 8], F32)

            for eb in range(NBLK):
                esl = slice(eb * EB, (eb + 1) * EB)
                xt = xin_pool.tile([128, 8, EB], FP8, tag="xt")
                # Split the first block's DMA by contraction halves so the
                # first matmuls can start as soon as k-chunks 0..3 land.
                if eb == 0:
                    nc.sync.dma_start(out=xt[:, 0:4], in_=extT_r[:, 0:4, esl])
                    nc.sync.dma_start(out=xt[:, 4:8], in_=extT_r[:, 4:8, esl])
                else:
                    nc.sync.dma_start(out=xt, in_=extT_r[:, :, esl])
                # A PE matmul may carry at most one semaphore wait (walrus
                # S3_LW limit).  Consume the xt-DMA dependencies with tiny
                # throwaway matmuls so the real matmuls only ever wait on one
                # source (the PSUM tile WAR release).
                nc.tensor.matmul(dum, lhsT=xt[:, 0, 0:128], rhs=xt[:, 0, 0:8],
                                 start=True, stop=True, skip_group_check=True)
                for m in range(2):
                    ps = ps_pool.tile([128, EB], F32, tag="ps", name="ps")
                    for k2 in range(4):
                        if eb == 0 and m == 0 and k2 == 2:
                            nc.tensor.matmul(dum, lhsT=xt[:, 4, 0:128],
                                             rhs=xt[:, 4, 0:8],
                                             start=True, stop=True,
                                             skip_group_check=True)
                        nc.tensor.matmul(
                            ps,
                            lhsT=refT_sb[:, 2 * k2:2 * k2 + 2,
                                         m * 128:(m + 1) * 128],
                            rhs=xt[:, 2 * k2:2 * k2 + 2, :],
                            start=(k2 == 0), stop=(k2 == 3),
                            perf_mode=DR,
                        )
                    nc.vector.max(out=pk[:, m, eb, 0:8].bitcast(F32), in_=ps)
                    nc.vector.max_index(out=pk[:, m, eb, 8:16],
                                        in_max=pk[:, m, eb, 0:8].bitcast(F32),
                                        in_values=ps)
                nc.sync.dma_start(out=out_o[:, :, eb, :], in_=pk[:, :, eb, :])
    nc.compile()
    return nc


def _build_phase2():
    # Transposed layout: partition axis = d (NC4 chunks of 128), free = rows.
    # Input S slot order: rt=0, rs=1, sht=2..4, simh=5..8, shs=9..11.
    # Epos slots: rt=0, rs=1, sht=2..4 (instr a = exp S[0:5]),
    #             shs=5..7           (instr b = exp S[9:12])
    # Eneg slots (exp of -S[2:12]): sht=0..2, simh=3..6, shs=7..9
    nc = bacc.Bacc("TRN2", target_bir_lowering=False, debug=False,
                   enable_asserts=False, num_devices=N_CORES)
    S_in = nc.dram_tensor("sin", (128, NC4, 12, 128), FP16,
                          kind="ExternalInput").ap()
    ones_in = nc.dram_tensor("ones", (128, 1), FP16, kind="ExternalInput").ap()
    ZB_o = nc.dram_tensor("zb", (128, NC4, 3 * N_UNITS), F32,
                          kind="ExternalOutput").ap()

    Exp = mybir.ActivationFunctionType.Exp

    with TileContext(nc) as tc:
        with (
            tc.tile_pool(name="src", bufs=1) as src_pool,
            tc.tile_pool(name="zps", bufs=1, space="PSUM") as zps_pool,
        ):
            ones = src_pool.tile([128, 1], FP16)
            nc.sync.dma_start(out=ones, in_=ones_in)

            for c in range(NC4):
                S = src_pool.tile([128, 12, 128], FP16, tag=f"S{c}")
                nc.sync.dma_start(out=S, in_=S_in[:, c])

                Epos = src_pool.tile([128, 8, 128], FP16, tag=f"Ep{c}")
                nc.scalar.activation(Epos[:, 0:5], S[:, 0:5], Exp)
                nc.scalar.activation(Epos[:, 5:8], S[:, 9:12], Exp)
                Eneg = src_pool.tile([128, 10, 128], FP16, tag=f"En{c}")
                nc.scalar.activation(Eneg, S[:, 2:12], Exp, scale=-1.0)

                # base diffs for dap = xt - xs (on the Pool engine)
                rd = src_pool.tile([128, 1, 128], FP16, tag=f"rd{c}")
                nc.gpsimd.tensor_sub(rd, S[:, 0:1], S[:, 1:2])
                sd = src_pool.tile([128, 3, 128], FP16, tag=f"sd{c}")
                nc.gpsimd.tensor_sub(sd, S[:, 2:5], S[:, 9:12])
                dd1 = src_pool.tile([128, 3, 128], FP16, tag=f"dd{c}")
                nc.gpsimd.tensor_sub(dd1, rd.broadcast_to((128, 3, 128)), sd)

                def b3(ap, n):  # broadcast a [128,1,r] slice over n units
                    return ap.broadcast_to((128, n, 128))

                # products; Pt/W unit order: d1 j0..2, d2 k0..3, d3 (j,k)
                #           Ps   unit order: d2 k0..3, d1 j0..2, d3 (j,k)
                Pt = src_pool.tile([128, N_UNITS, 128], FP16, tag=f"Pt{c}")
                nc.vector.tensor_mul(Pt[:, 0:7], b3(Epos[:, 0:1], 7),
                                     Eneg[:, 0:7])
                sht_e = Epos[:, 2:5].rearrange("p j (o r) -> p j o r", o=1) \
                    .broadcast_to((128, 3, 4, 128))
                simh_e = Eneg[:, 3:7].rearrange("p (o k) r -> p o k r", o=1) \
                    .broadcast_to((128, 3, 4, 128))
                Pt_d3 = Pt[:, 7:19].rearrange("p (j k) r -> p j k r", j=3)
                nc.vector.tensor_mul(Pt_d3, sht_e, simh_e)

                Ps = src_pool.tile([128, N_UNITS, 128], FP16, tag=f"Qs{c}")
                nc.vector.tensor_mul(Ps[:, 0:7], b3(Epos[:, 1:2], 7),
                                     Eneg[:, 3:10])
                shs_e = Epos[:, 5:8].rearrange("p j (o r) -> p j o r", o=1) \
                    .broadcast_to((128, 3, 4, 128))
                Ps_d3 = Ps[:, 7:19].rearrange("p (j k) r -> p j k r", j=3)
                nc.vector.tensor_mul(Ps_d3, shs_e, simh_e)

                W = src_pool.tile([128, N_UNITS, 128], FP16, tag=f"W{c}")
                nc.vector.tensor_mul(W[:, 0:3], Pt[:, 0:3], dd1)
                nc.vector.tensor_mul(W[:, 3:7], Pt[:, 3:7], b3(rd, 4))
                sd_e = sd.rearrange("p j (o r) -> p j o r", o=1) \
                    .broadcast_to((128, 3, 4, 128))
                W_d3 = W[:, 7:19].rearrange("p (j k) r -> p j k r", j=3)
                nc.vector.tensor_mul(W_d3, Pt_d3, sd_e)

                # All 57 reductions over this chunk's d on the PE:
                # ones-vector matmuls, one PSUM column each (start+stop).
                Zc = zps_pool.tile([128, 3 * N_UNITS], F32, tag=f"Z{c}")
                for t, T in enumerate((Pt, Ps, W)):
                    for u in range(N_UNITS):
                        col = t * N_UNITS + u
                        nc.tensor.matmul(
                            Zc[:, col:col + 1],
                            lhsT=T[:, u, :],
                            rhs=ones,
                            start=True, stop=True,
                            skip_group_check=True,
                        )
                zsb = src_pool.tile([128, 3 * N_UNITS], F32, tag=f"zs{c}")
                nc.scalar.copy(zsb, Zc)
                nc.sync.dma_start(out=ZB_o[:, c], in_=zsb)
    nc.compile()
    return nc


def _get(name):
    if name not in _CACHE:
        _CACHE[name] = _build_phase1() if name == "p1" else _build_phase2()
    return _CACHE[name]


def kernel(**inputs):
    tf = np.ascontiguousarray(np.asarray(inputs["teacher_feats"], dtype=np.float32))
    sf = np.ascontiguousarray(np.asarray(inputs["student_feats"], dtype=np.float32))
    in_dtype = np.asarray(inputs["ref_perm"]).dtype
    ref_perm = np.asarray(inputs["ref_perm"]).astype(np.int64)[:NUM_REF]
    shared_perm = np.asarray(inputs["shared_perm"]).astype(np.int64)[:NUM_SHARED]
    assert in_dtype == np.int32

    # ---- host gathers + normalization (tiny) ----
    ref_t = tf[:, 0, ref_perm, :]                       # [B, 256, 1024]
    ref_s = sf[:, 0, ref_perm, :]
    rn = np.sqrt(np.einsum("brd,brd->br", ref_t, ref_t))[..., None]
    refn = ref_t / np.maximum(rn, 1e-12)
    # scale by 64 so fp8e4m3 operates in its normal range
    refTs = [np.ascontiguousarray((refn[b].T * 64.0).astype(FP8_NP))
             for b in range(B)]

    # ---- phase 1: sharded cosine-sim + per-block top-8 ----
    in_maps1 = []
    for c in range(N_CORES):
        b, s = divmod(c, 4)
        x = tf[b, EXTRA_FRAMES[s]]                      # [4096, 1024]
        nrm = np.sqrt(np.einsum("ed,ed->e", x, x))
        xn = x / np.maximum(nrm, 1e-12)[:, None]
        extT = np.ascontiguousarray((xn.T * 64.0).astype(FP8_NP))
        in_maps1.append({"extT": extT, "refT": refTs[b]})

    res1 = bass_utils.run_bass_kernel_spmd(
        _get("p1"), in_maps1, core_ids=list(range(N_CORES)))
    LAST_PERF["p1"] = res1

    # ---- host cross-block/cross-shard top-k merge ----
    gidx = np.zeros((B, NUM_REF, TOPK), dtype=np.int64)
    for b in range(B):
        vals_l, idxs_l = [], []
        for s in range(4):
            raw = np.ascontiguousarray(res1.results[b * 4 + s]["out"])
            vals = raw[..., 0:8].view(np.float32)            # [128,2,NBLK,8]
            bidx = raw[..., 8:16].astype(np.int64)
            gl = (bidx + (np.arange(NBLK) * EB)[None, None, :, None]
                  + s * ES)
            # row r = m*128 + p
            vals_l.append(vals.transpose(1, 0, 2, 3).reshape(NUM_REF, -1))
            idxs_l.append(gl.transpose(1, 0, 2, 3).reshape(NUM_REF, -1))
        vals = np.concatenate(vals_l, axis=1)
        idxs = np.concatenate(idxs_l, axis=1)
        order = np.argsort(-vals, axis=1, kind="stable")[:, :TOPK]
        gidx[b] = np.take_along_axis(idxs, order, axis=1)

    fr = np.asarray(EXTRA_FRAMES, dtype=np.int64)[gidx // P]
    pt = gidx % P
    sim_high = tf[np.arange(B)[:, None, None], fr, pt]  # [B, 256, 4, 1024]

    # ---- phase 2: distances ----
    sh_t = np.stack([tf[:, t, shared_perm, :] for t in SHARED_TEACHER], axis=1)
    sh_s = np.stack([sf[:, s, shared_perm, :] for s in SHARED_STUDENT], axis=1)

    def t_chunks(a):  # [128 rows, DH] -> [128 dpart, NC4, 128 rows]
        return a.T.reshape(NC4, 128, 128).transpose(1, 0, 2)

    ones_arr = np.ones((128, 1), dtype=np.float16)
    in_maps2 = []
    for c in range(N_CORES):
        b, h, dh = c >> 2, (c >> 1) & 1, c & 1
        rs = slice(h * 128, (h + 1) * 128)
        cs = slice(dh * DH, (dh + 1) * DH)
        # S slot order: rt, rs, sht0..2, simh0..3, shs0..2
        slots = ([ref_t[b, rs, cs], ref_s[b, rs, cs]]
                 + [sh_t[b, j, rs, cs] for j in range(3)]
                 + [sim_high[b, rs, k, cs] for k in range(4)]
                 + [sh_s[b, j, rs, cs] for j in range(3)])
        S = np.stack([t_chunks(a) for a in slots], axis=2)  # [128,NC4,12,128]
        in_maps2.append({"sin": np.ascontiguousarray(S.astype(np.float16)),
                         "ones": ones_arr})

    res2 = bass_utils.run_bass_kernel_spmd(
        _get("p2"), in_maps2, core_ids=list(range(N_CORES)))
    LAST_PERF["p2"] = res2

    # ---- host tail: kl + SmoothL1 + averaging ----
    s1 = s2 = s3 = 0.0
    for b in range(B):
        for h in range(2):
            z = (res2.results[b * 4 + h * 2 + 0]["zb"].astype(np.float64)
                 + res2.results[b * 4 + h * 2 + 1]["zb"].astype(np.float64)
                 ).sum(axis=1)
            Zt = z[:, 0:N_UNITS]
            Zs = z[:, N_UNITS:2 * N_UNITS][:, ZS_PERM]
            num = z[:, 2 * N_UNITS:3 * N_UNITS]          # [128, 19]
            kl = num / Zt - np.log(Zt) + np.log(Zs)
            akl = np.abs(kl)
            hub = np.where(akl < BETA, 0.5 * kl * kl / BETA, akl - 0.5 * BETA)
            s1 += hub[:, 0:3].sum()
            s2 += hub[:, 3:7].sum()
            s3 += hub[:, 7:19].sum()

    loss = (s1 / (3 * B * NUM_REF)
            + s2 / (B * NUM_REF * TOPK)
            + s3 / (3 * B * NUM_REF * TOPK))
    return np.float32(loss)


# revision 34
# speedup vs baseline: 1.0780x; 1.0075x over previous
"""Trainium2 Bass kernel for nn_DA3CrossFrameCFDistanceLoss.

Strategy (8 NeuronCores):
  Phase 1 (data-parallel over batch x extra-frame shard):
    core c -> (b = c//4, shard s = c%4).  Host pre-normalizes both the ref
    rows and the extra-frame rows (folding the 1/||x|| scaling into the
    operands), scales by 64 and ships them as fp8e4m3, quartering HBM
    traffic.  Each core streams its frame transposed (extT [D, 4096] fp8)
    in 512-column blocks and runs DoubleRow fp8 matmuls (256-deep
    contraction per instruction, 2x PE rate) against the stationary ref
    block with fp32 PSUM accumulation.  Per block the DVE extracts top-8
    values + their indices per ref row directly from PSUM (max/max_index)
    and packs them (f32 bits + uint32) into one output row, DMA'd out per
    block.  The host merges the 8 blocks x 4 shards top-8s per row into
    the global top-4 and gathers the selected rows at full fp32 precision.
    (Selection is approximate only through the fp8 rounding of the
    similarity inputs - validated to move the final loss by ~5e-5
    relative against the 2e-2 tolerance.)
  Phase 2 (data-parallel over (batch, row-half, feature-half)):
    core c -> (b, h, dh).  Layout is transposed on host: d sits on the
    partition axis (4 chunks of 128), rows on the free axis.  The 19 KL
    "units" need Zt = sum exp(xt), Zs = sum exp(xs), num = sum exp(xt)*
    (xt - xs).  Using exp(a-b) = exp(a)*exp(-b), the ACT engine computes
    just 18 base exponentials (3 instructions per chunk); the DVE forms
    the per-unit products Pt, Ps and W = Pt*(xt-xs) as wide
    stride-0-broadcast tensor ops (7 instructions per chunk); the Pool
    engine supplies the base diffs; and the otherwise-idle PE performs
    all 57 per-unit reductions over d as ones-vector matmuls, one PSUM
    column per (chunk, tensor, unit) so no cross-instruction PSUM
    accumulation is needed (the bank-granular pending-zero of
    start_tensor_calc makes interleaved accumulation groups unsafe).
    Per-chunk results are copied and DMA'd out as soon as they finish;
    the host sums the chunk partials and evaluates kl = num/Zt - log Zt
    + log Zs, SmoothL1 and the weighted averaging.
"""

import numpy as np
import ml_dtypes

import concourse.bass as bass
from concourse import bacc
import concourse.mybir as mybir
from concourse import bass_utils
from concourse.tile import TileContext

# ---- problem constants (hardcoded from the nn.Module defaults) ----
B, V, P, D = 2, 8, 4096, 1024
EXTRA_FRAMES = [1, 3, 5, 7]
SHARED_TEACHER = [2, 4, 6]
SHARED_STUDENT = [1, 2, 3]
NUM_REF = 256
NUM_SHARED = 256
TOPK = 4
TEMP = 1.0
BETA = 0.5
N_CORES = 8

ES = P          # extra rows per shard (one frame per shard)
EB = 512        # phase-1 e-block size (one PSUM bank of fp32)
NBLK = ES // EB
# phase-1 compute blocks (start, width): block 0 split for faster rampup
BLOCKS = [(0, 256), (256, 256)] + [(i * EB, EB) for i in range(1, NBLK)]
NB_OUT = len(BLOCKS)
DH = D // 2     # phase-2 feature half
NC4 = DH // 128  # phase-2 d chunks per core
N_UNITS = 19    # 3 d1 + 4 d2 + 12 d3

F32 = mybir.dt.float32
BF16 = mybir.dt.bfloat16
FP16 = mybir.dt.float16
FP8 = mybir.dt.float8e4
U16 = mybir.dt.uint16
U32 = mybir.dt.uint32

BF16_NP = ml_dtypes.bfloat16
FP8_NP = mybir.dt.np(FP8)

# phase-2 Zs unit permutation (Ps is built d2-first for AP adjacency)
ZS_PERM = [4, 5, 6, 0, 1, 2, 3] + list(range(7, 19))

_CACHE = {}

# Results of the most recent launches (exec_time_ns etc), for test harnesses.
LAST_PERF = {}


def _build_phase1():
    nc = bacc.Bacc("TRN2", target_bir_lowering=False, debug=False,
                   enable_asserts=False, num_devices=N_CORES)
    extT = nc.dram_tensor("extT", (D, ES), FP8, kind="ExternalInput").ap()
    refT = nc.dram_tensor("refT", (D, NUM_REF), FP8, kind="ExternalInput").ap()
    # packed per-block output: cols 0:8 top-8 values (f32 bits),
    # cols 8:16 their uint32 in-block indices
    out_o = nc.dram_tensor("out", (128, 2, NB_OUT, 16), U32,
                           kind="ExternalOutput").ap()

    extT_r = extT.rearrange("(k p) e -> p k e", p=128)
    refT_r = refT.rearrange("(k p) r -> p k r", p=128)
    DR = mybir.MatmulPerfMode.DoubleRow

    with TileContext(nc) as tc:
        with (
            tc.tile_pool(name="const", bufs=1) as const_pool,
            tc.tile_pool(name="xin", bufs=3) as xin_pool,
            tc.tile_pool(name="ps", bufs=6, space="PSUM") as ps_pool,
            tc.tile_pool(name="dum", bufs=1, space="PSUM") as dum_pool,
            tc.tile_pool(name="small", bufs=1) as small_pool,
        ):
            refT_sb = const_pool.tile([128, 8, NUM_REF], FP8)
            nc.sync.dma_start(out=refT_sb, in_=refT_r)
            pk = small_pool.tile([128, 2, NB_OUT, 16], U32)
            dum = dum_pool.tile([128, 8], F32)

            # Block 0 is split into two 256-column sub-blocks (by columns)
            # so the DVE top-8 chain - the rate-limiting engine - starts as
            # soon as the first quarter-MB of extT lands.
            for ob, (est, ew) in enumerate(BLOCKS):
                if est % EB == 0:
                    xt = xin_pool.tile([128, 8, EB], FP8, tag="xt")
                    nc.sync.dma_start(out=xt[:, :, 0:ew],
                                      in_=extT_r[:, :, est:est + ew])
                    if ew < EB:
                        nc.sync.dma_start(out=xt[:, :, ew:EB],
                                          in_=extT_r[:, :, est + ew:est + EB])
                    xo = 0
                else:
                    xo = est % EB
                # A PE matmul may carry at most one semaphore wait (walrus
                # S3_LW limit).  Consume the xt-DMA dependency with a tiny
                # throwaway matmul so the real matmuls only ever wait on one
                # source (the PSUM tile WAR release).
                nc.tensor.matmul(dum, lhsT=xt[:, 0, xo:xo + 128],
                                 rhs=xt[:, 0, xo:xo + 8],
                                 start=True, stop=True, skip_group_check=True)
                for m in range(2):
                    ps = ps_pool.tile([128, EB], F32, tag="ps", name="ps")
                    for k2 in range(4):
                        nc.tensor.matmul(
                            ps[:, 0:ew],
                            lhsT=refT_sb[:, 2 * k2:2 * k2 + 2,
                                         m * 128:(m + 1) * 128],
                            rhs=xt[:, 2 * k2:2 * k2 + 2, xo:xo + ew],
                            start=(k2 == 0), stop=(k2 == 3),
                            perf_mode=DR,
                        )
                    nc.vector.max(out=pk[:, m, ob, 0:8].bitcast(F32),
                                  in_=ps[:, 0:ew])
                    nc.vector.max_index(out=pk[:, m, ob, 8:16],
                                        in_max=pk[:, m, ob, 0:8].bitcast(F32),
                                        in_values=ps[:, 0:ew])
                nc.sync.dma_start(out=out_o[:, :, ob, :], in_=pk[:, :, ob, :])
    nc.compile()
    return nc


def _build_phase2():
    # Transposed layout: partition axis = d (NC4 chunks of 128), free = rows.
    # Input S slot order: rt=0, rs=1, sht=2..4, simh=5..8, shs=9..11.
    # Epos slots: rt=0, rs=1, sht=2..4 (instr a = exp S[0:5]),
    #             shs=5..7           (instr b = exp S[9:12])
    # Eneg slots (exp of -S[2:12]): sht=0..2, simh=3..6, shs=7..9
    nc = bacc.Bacc("TRN2", target_bir_lowering=False, debug=False,
                   enable_asserts=False, num_devices=N_CORES)
    S_in = nc.dram_tensor("sin", (128, NC4, 12, 128), FP16,
                          kind="ExternalInput").ap()
    ZB_o = nc.dram_tensor("zb", (128, NC4, 3 * N_UNITS), F32,
                          kind="ExternalOutput").ap()

    Exp = mybir.ActivationFunctionType.Exp

    with TileContext(nc) as tc:
        with (
            tc.tile_pool(name="src", bufs=1) as src_pool,
            tc.tile_pool(name="zps", bufs=1, space="PSUM") as zps_pool,
        ):
            ones = src_pool.tile([128, 1], FP16)
            nc.gpsimd.memset(ones, 1.0)

            for c in range(NC4):
                S = src_pool.tile([128, 12, 128], FP16, tag=f"S{c}")
                nc.sync.dma_start(out=S, in_=S_in[:, c])

                Epos = src_pool.tile([128, 8, 128], FP16, tag=f"Ep{c}")
                nc.scalar.activation(Epos[:, 0:5], S[:, 0:5], Exp)
                nc.scalar.activation(Epos[:, 5:8], S[:, 9:12], Exp)
                Eneg = src_pool.tile([128, 10, 128], FP16, tag=f"En{c}")
                nc.scalar.activation(Eneg, S[:, 2:12], Exp, scale=-1.0)

                # base diffs for dap = xt - xs (on the Pool engine)
                rd = src_pool.tile([128, 1, 128], FP16, tag=f"rd{c}")
                nc.gpsimd.tensor_sub(rd, S[:, 0:1], S[:, 1:2])
                sd = src_pool.tile([128, 3, 128], FP16, tag=f"sd{c}")
                nc.gpsimd.tensor_sub(sd, S[:, 2:5], S[:, 9:12])
                dd1 = src_pool.tile([128, 3, 128], FP16, tag=f"dd{c}")
                nc.gpsimd.tensor_sub(dd1, rd.broadcast_to((128, 3, 128)), sd)

                def b3(ap, n):  # broadcast a [128,1,r] slice over n units
                    return ap.broadcast_to((128, n, 128))

                # products; Pt/W unit order: d1 j0..2, d2 k0..3, d3 (j,k)
                #           Ps   unit order: d2 k0..3, d1 j0..2, d3 (j,k)
                Pt = src_pool.tile([128, N_UNITS, 128], FP16, tag=f"Pt{c}")
                nc.vector.tensor_mul(Pt[:, 0:7], b3(Epos[:, 0:1], 7),
                                     Eneg[:, 0:7])
                sht_e = Epos[:, 2:5].rearrange("p j (o r) -> p j o r", o=1) \
                    .broadcast_to((128, 3, 4, 128))
                simh_e = Eneg[:, 3:7].rearrange("p (o k) r -> p o k r", o=1) \
                    .broadcast_to((128, 3, 4, 128))
                Pt_d3 = Pt[:, 7:19].rearrange("p (j k) r -> p j k r", j=3)
                nc.vector.tensor_mul(Pt_d3, sht_e, simh_e)

                Ps = src_pool.tile([128, N_UNITS, 128], FP16, tag=f"Qs{c}")
                nc.vector.tensor_mul(Ps[:, 0:7], b3(Epos[:, 1:2], 7),
                                     Eneg[:, 3:10])
                shs_e = Epos[:, 5:8].rearrange("p j (o r) -> p j o r", o=1) \
                    .broadcast_to((128, 3, 4, 128))
                Ps_d3 = Ps[:, 7:19].rearrange("p (j k) r -> p j k r", j=3)
                nc.vector.tensor_mul(Ps_d3, shs_e, simh_e)

                W = src_pool.tile([128, N_UNITS, 128], FP16, tag=f"W{c}")
                nc.vector.tensor_mul(W[:, 0:3], Pt[:, 0:3], dd1)
                nc.vector.tensor_mul(W[:, 3:7], Pt[:, 3:7], b3(rd, 4))
                sd_e = sd.rearrange("p j (o r) -> p j o r", o=1) \
                    .broadcast_to((128, 3, 4, 128))
                W_d3 = W[:, 7:19].rearrange("p (j k) r -> p j k r", j=3)
                nc.vector.tensor_mul(W_d3, Pt_d3, sd_e)

                # All 57 reductions over this chunk's d on the PE:
                # ones-vector matmuls, one PSUM column each (start+stop).
                Zc = zps_pool.tile([128, 3 * N_UNITS], F32, tag=f"Z{c}")
                for t, T in enumerate((Pt, Ps, W)):
                    for u in range(N_UNITS):
                        col = t * N_UNITS + u
                        nc.tensor.matmul(
                            Zc[:, col:col + 1],
                            lhsT=T[:, u, :],
                            rhs=ones,
                            start=True, stop=True,
                            skip_group_check=True,
                        )
                zsb = src_pool.tile([128, 3 * N_UNITS], F32, tag=f"zs{c}")
                nc.scalar.copy(zsb, Zc)
                nc.sync.dma_start(out=ZB_o[:, c], in_=zsb)
    nc.compile()
    return nc


def _get(name):
    if name not in _CACHE:
        _CACHE[name] = _build_phase1() if name == "p1" else _build_phase2()
    return _CACHE[name]


def kernel(**inputs):
    tf = np.ascontiguousarray(np.asarray(inputs["teacher_feats"], dtype=np.float32))
    sf = np.ascontiguousarray(np.asarray(inputs["student_feats"], dtype=np.float32))
    in_dtype = np.asarray(inputs["ref_perm"]).dtype
    ref_perm = np.asarray(inputs["ref_perm"]).astype(np.int64)[:NUM_REF]
    shared_perm = np.asarray(inputs["shared_perm"]).astype(np.int64)[:NUM_SHARED]
    assert in_dtype == np.int32

    # ---- host gathers + normalization (tiny) ----
    ref_t = tf[:, 0, ref_perm, :]                       # [B, 256, 1024]
    ref_s = sf[:, 0, ref_perm, :]
    rn = np.sqrt(np.einsum("brd,brd->br", ref_t, ref_t))[..., None]
    refn = ref_t / np.maximum(rn, 1e-12)
    # scale by 64 so fp8e4m3 operates in its normal range
    refTs = [np.ascontiguousarray((refn[b].T * 64.0).astype(FP8_NP))
             for b in range(B)]

    # ---- phase 1: sharded cosine-sim + per-block top-8 ----
    in_maps1 = []
    for c in range(N_CORES):
        b, s = divmod(c, 4)
        x = tf[b, EXTRA_FRAMES[s]]                      # [4096, 1024]
        nrm = np.sqrt(np.einsum("ed,ed->e", x, x))
        xn = x / np.maximum(nrm, 1e-12)[:, None]
        extT = np.ascontiguousarray((xn.T * 64.0).astype(FP8_NP))
        in_maps1.append({"extT": extT, "refT": refTs[b]})

    res1 = bass_utils.run_bass_kernel_spmd(
        _get("p1"), in_maps1, core_ids=list(range(N_CORES)))
    LAST_PERF["p1"] = res1

    # ---- host cross-block/cross-shard top-k merge ----
    gidx = np.zeros((B, NUM_REF, TOPK), dtype=np.int64)
    for b in range(B):
        vals_l, idxs_l = [], []
        for s in range(4):
            raw = np.ascontiguousarray(res1.results[b * 4 + s]["out"])
            vals = raw[..., 0:8].view(np.float32)          # [128,2,NB_OUT,8]
            bidx = raw[..., 8:16].astype(np.int64)
            starts = np.asarray([blk[0] for blk in BLOCKS], dtype=np.int64)
            gl = bidx + starts[None, None, :, None] + s * ES
            # row r = m*128 + p
            vals_l.append(vals.transpose(1, 0, 2, 3).reshape(NUM_REF, -1))
            idxs_l.append(gl.transpose(1, 0, 2, 3).reshape(NUM_REF, -1))
        vals = np.concatenate(vals_l, axis=1)
        idxs = np.concatenate(idxs_l, axis=1)
        order = np.argsort(-vals, axis=1, kind="stable")[:, :TOPK]
        gidx[b] = np.take_along_axis(idxs, order, axis=1)

    fr = np.asarray(EXTRA_FRAMES, dtype=np.int64)[gidx // P]
    pt = gidx % P
    sim_high = tf[np.arange(B)[:, None, None], fr, pt]  # [B, 256, 4, 1024]

    # ---- phase 2: distances ----
    sh_t = np.stack([tf[:, t, shared_perm, :] for t in SHARED_TEACHER], axis=1)
    sh_s = np.stack([sf[:, s, shared_perm, :] for s in SHARED_STUDENT], axis=1)

    def t_chunks(a):  # [128 rows, DH] -> [128 dpart, NC4, 128 rows]
        return a.T.reshape(NC4, 128, 128).transpose(1, 0, 2)

    in_maps2 = []
    for c in range(N_CORES):
        b, h, dh = c >> 2, (c >> 1) & 1, c & 1
        rs = slice(h * 128, (h + 1) * 128)
        cs = slice(dh * DH, (dh + 1) * DH)
        # S slot order: rt, rs, sht0..2, simh0..3, shs0..2
        slots = ([ref_t[b, rs, cs], ref_s[b, rs, cs]]
                 + [sh_t[b, j, rs, cs] for j in range(3)]
                 + [sim_high[b, rs, k, cs] for k in range(4)]
                 + [sh_s[b, j, rs, cs] for j in range(3)])
        S = np.stack([t_chunks(a) for a in slots], axis=2)  # [128,NC4,12,128]
        in_maps2.append({"sin": np.ascontiguousarray(S.astype(np.float16))})

    res2 = bass_utils.run_bass_kernel_spmd(
        _get("p2"), in_maps2, core_ids=list(range(N_CORES)))
    LAST_PERF["p2"] = res2

    # ---- host tail: kl + SmoothL1 + averaging ----
    s1 = s2 = s3 = 0.0
    for b in range(B):
        for h in range(2):
            z = (res2.results[b * 4 + h * 2 + 0]["zb"].astype(np.float64)
                 + res2.results[b * 4 + h * 2 + 1]["zb"].astype(np.float64)
                 ).sum(axis=1)
            Zt = z[:, 0:N_UNITS]
            Zs = z[:, N_UNITS:2 * N_UNITS][:, ZS_PERM]
            num = z[:, 2 * N_UNITS:3 * N_UNITS]          # [128, 19]
            kl = num / Zt - np.log(Zt) + np.log(Zs)
            akl = np.abs(kl)
            hub = np.where(akl < BETA, 0.5 * kl * kl / BETA, akl - 0.5 * BETA)
            s1 += hub[:, 0:3].sum()
            s2 += hub[:, 3:7].sum()
            s3 += hub[:, 7:19].sum()

    loss = (s1 / (3 * B * NUM_REF)
            + s2 / (B * NUM_REF * TOPK)
            + s3 / (3 * B * NUM_REF * TOPK))
    return np.float32(loss)
